# revision 4
# baseline (speedup 1.0000x reference)
"""Bass/Tile kernel v2 for nn_CrossAttention (retrieval_knn):
out = softmax(-cdist(Q, K) / 8, axis=-1), Q/K: [4, 4096, 64] fp32.

Sharding: 16384 query rows across 8 cores (2048 rows/core); K replicated
per batch (cores 2b, 2b+1 get K[b]).

v2 design (cost-model driven):
  PE:   psum[n,m] = qk - k2/2, single f32r matmul (K=65, ones row carries
        -k2/2 precomputed on host).                                ~27 us
  ACT:  s = sqrt(-2c^2*psum + c^2*q2) -> fp16  (c = log2(e)/8, so
        s = c*dist; bias = c^2*q2 per partition, host-precomputed) ~55 us
  ACT:  e = exp(-ln2 * s) = 2^-s, in-place fp16, accum_out row sums ~55 us
        (one sqrt->exp table switch per pass; ~1.3us each)
  DVE:  recs = 1/sums; out_bf16 = e * recs  (4x perf mode: 2-byte
        in/out, SBUF-only)                                         ~17 us
  DMA:  store [128, 4096] bf16 per row-tile (1 MiB)                ~51 us
  Host: unpack bf16 -> fp32.
"""

import sys
import numpy as np

try:
    import concourse.bass as bass  # noqa: F401
except ImportError:  # container staging path
    sys.path.insert(0, "/opt/trn_rl_repo")
    import concourse.bass as bass  # noqa: F401

import concourse.mybir as mybir
import concourse.tile as tile
from concourse import bacc
from concourse.bass import ts
from concourse.bass_utils import run_bass_kernel_spmd
from concourse.tile import add_dep_helper

F32 = mybir.dt.float32
F32R = mybir.dt.float32r
F16 = mybir.dt.float16
BF16 = mybir.dt.bfloat16
AF = mybir.ActivationFunctionType

B, N, M, D = 4, 4096, 4096, 64
N_CORES = 8
ROWS = B * N // N_CORES  # 2048 query rows per core

C_SCALE = float(np.log2(np.e) / 8.0)  # s~ = C_SCALE * dist
LN2 = float(np.log(2.0))

DEFAULT_KW = dict(groups=(16,), ch=2048)


def round_f32r(x):
    """fp32 -> fp32r rounding (RNE at mantissa bit 12), matching the PE."""
    u = np.ascontiguousarray(x, np.float32).view(np.uint32)
    lo = u & np.uint32(0xFFF)
    hi = u & np.uint32(0xFFFFF000)
    up = (lo > 0x800) | ((lo == 0x800) & (((u >> np.uint32(12)) & np.uint32(1)) == 1))
    return (hi + np.where(up, np.uint32(0x1000), np.uint32(0))).view(np.float32)


def build_kernel(rows=ROWS, m=M, ch=2048, groups=(8, 8), reps=1,
                 s_dtype=F16, out_dtype=BF16, store_q="sync", load_q="gpsimd",
                 no_norm=False, no_store=False, no_exp=False,
                 warmup_mm=0, fine_first=0, early_tload=True, tail_piece=0,
                 tail_norm=4, dve_sums=False, mm_width=512):
    assert rows % 128 == 0 and m % 512 == 0 and ch % 512 == 0 and m % ch == 0
    n_tiles = rows // 128
    n_ch = m // ch
    mm_per_ch = ch // 512
    groups = list(groups)
    assert sum(groups) == n_tiles

    nc = bacc.Bacc("TRN2", target_bir_lowering=False, debug=False)
    qt = nc.dram_tensor("qt", [D + 1, rows], F32R, kind="ExternalInput")
    kt = nc.dram_tensor("kt", [D + 1, m], F32R, kind="ExternalInput")
    q2c = nc.dram_tensor("q2c", [128, n_tiles], F32, kind="ExternalInput")
    out = nc.dram_tensor("out", [rows, m], out_dtype, kind="ExternalOutput")

    with tile.TileContext(nc) as tc:
        with (
            tc.tile_pool(name="const", bufs=1) as cpool,
            tc.tile_pool(name="spool", bufs=max(groups)) as spool,
            tc.tile_pool(name="opool", bufs=4) as opool,
            tc.tile_pool(name="psum", bufs=2, space="PSUM") as ppool,
        ):
          for _rep in range(reps):
            # ---- prologue: loads ----
            if early_tload:
                # dummy dep-free sqrt so the auto-inserted sqrt-table load
                # runs during the input loads, off the critical path
                dz = cpool.tile([128, 1], F32, name="dz")
                nc.vector.memset(dz[:, :], 1.0)
                nc.scalar.activation(out=dz[:, :], in_=dz[:, :], func=AF.Sqrt)
            # critical path: qe cols 0-127 + ke cols 0-511 unblock matmul 0.
            qe = cpool.tile([D + 1, rows], F32R, name="qe")
            ke = cpool.tile([D + 1, m], F32R, name="ke")
            q2t = cpool.tile([128, n_tiles], F32, name="q2t")
            lq = nc.sync if load_q == "sync" else nc.gpsimd
            lq2 = nc.gpsimd if load_q == "sync" else nc.sync
            lq2.dma_start(out=qe[:, 0:128], in_=qt[:, 0:128])
            lq2.dma_start(out=q2t[:, :], in_=q2c[:, :])
            for c in range(4):  # K chunks split across both trigger queues
                lq.dma_start(out=ke[:, ts(2 * c, 512)],
                             in_=kt[:, ts(2 * c, 512)])
                lq2.dma_start(out=ke[:, ts(2 * c + 1, 512)],
                              in_=kt[:, ts(2 * c + 1, 512)])
            lq.dma_start(out=qe[:, 128:1024], in_=qt[:, 128:1024])
            lq2.dma_start(out=qe[:, 1024:], in_=qt[:, 1024:])

            sums = cpool.tile([128, n_tiles], F32, name="sums")
            recs = cpool.tile([128, n_tiles], F32, name="recs")

            if warmup_mm:
                # ramp the PE pstate clock during the load prologue with
                # dummy matmuls on a zeroed tile (output never read)
                wz = cpool.tile([D + 1, 512], F32R, name="wz")
                nc.vector.memset(wz[:, :].bitcast(F32), 0.0)
                wp = ppool.tile([128, ch], F32, tag="pm", name="wp")
                for _ in range(warmup_mm):
                    nc.tensor.matmul(wp[:, 0:512], wz[:, 0:128], wz[:, :],
                                     start=True, stop=True)

            sq = nc.sync if store_q == "sync" else nc.gpsimd

            prev_last_exp = None
            g0 = 0
            for gsize in groups:
                gtiles = range(g0, g0 + gsize)
                g0 += gsize
                s_tiles = {}
                sqrt_acts = []
                for t in gtiles:  # sqrt phase
                    s_t = spool.tile([128, m], s_dtype, tag="s", name="s_t")
                    s_tiles[t] = s_t
                    for chi in range(n_ch):
                        pm = ppool.tile([128, ch], F32, tag="pm", name="pm")
                        fine = fine_first and t == 0 and chi == 0
                        w = 512 if fine else mm_width
                        for j in range(ch // w):
                            c = chi * (ch // w) + j
                            nc.tensor.matmul(
                                pm[:, ts(j, w)],
                                qe[:, ts(t, 128)],
                                ke[:, ts(c, w)],
                                start=True, stop=True,
                            )
                            if fine:
                                act = nc.scalar.activation(
                                    out=s_t[:, ts(j, 512)], in_=pm[:, ts(j, 512)],
                                    func=AF.Sqrt, scale=-2.0 * C_SCALE * C_SCALE,
                                    bias=q2t[:, t : t + 1],
                                )
                                sqrt_acts.append(act)
                        if fine:
                            continue
                        act = nc.scalar.activation(
                            out=s_t[:, ts(chi, ch)], in_=pm[:, :],
                            func=AF.Sqrt, scale=-2.0 * C_SCALE * C_SCALE,
                            bias=q2t[:, t : t + 1],
                        )
                        sqrt_acts.append(act)
                if prev_last_exp is not None:
                    for act in sqrt_acts:
                        add_dep_helper(act.ins, prev_last_exp.ins, False,
                                       "act-table phase order")
                if no_exp:
                    for t in gtiles:
                        if not no_store:
                            sq.dma_start(out=out[ts(t, 128), :],
                                         in_=s_tiles[t][:, :].bitcast(out_dtype))
                    continue
                last_t = gtiles[-1] if g0 == n_tiles else None
                psums = None
                exp_acts = []
                for t in gtiles:  # exp phase + row sums
                    if t == last_t and tail_piece > 1:
                        # last tile: piecewise exp/accum so the tail
                        # (combine+recip+norm+store) pipelines per piece
                        psums = cpool.tile([128, tail_piece], F32, name="psums")
                        pw = m // tail_piece
                        for p in range(tail_piece):
                            e = nc.scalar.activation(
                                out=s_tiles[t][:, ts(p, pw)],
                                in_=s_tiles[t][:, ts(p, pw)],
                                func=AF.Exp, scale=-LN2,
                                accum_out=psums[:, p : p + 1],
                            )
                            exp_acts.append(e)
                    elif dve_sums and t != last_t:
                        # row sums on the idle DVE; ACT skips the 187ns
                        # accumulator-read aux per instruction
                        e = nc.scalar.activation(
                            out=s_tiles[t][:, :], in_=s_tiles[t][:, :],
                            func=AF.Exp, scale=-LN2,
                        )
                        exp_acts.append(e)
                        nc.vector.tensor_reduce(
                            out=sums[:, t : t + 1], in_=s_tiles[t][:, :],
                            axis=mybir.AxisListType.X, op=mybir.AluOpType.add,
                        )
                    else:
                        e = nc.scalar.activation(
                            out=s_tiles[t][:, :], in_=s_tiles[t][:, :],
                            func=AF.Exp, scale=-LN2, accum_out=sums[:, t : t + 1],
                        )
                        exp_acts.append(e)
                for e in exp_acts:
                    add_dep_helper(e.ins, sqrt_acts[-1].ins, False,
                                   "act-table phase order")
                prev_last_exp = exp_acts[-1]
                for t in gtiles:  # normalize + store
                    if no_norm:
                        if not no_store:
                            sq.dma_start(out=out[ts(t, 128), :],
                                         in_=s_tiles[t][:, :].bitcast(out_dtype))
                        continue
                    if psums is not None and t == last_t:
                        nc.vector.tensor_reduce(
                            out=sums[:, t : t + 1], in_=psums[:, :],
                            axis=mybir.AxisListType.X, op=mybir.AluOpType.add,
                        )
                    nc.vector.reciprocal(out=recs[:, t : t + 1],
                                         in_=sums[:, t : t + 1])
                    o_t = opool.tile([128, m], out_dtype, tag="o", name="o_t")
                    np_pieces = tail_piece if (t == last_t and tail_piece > 1) \
                        else (tail_norm if (t == last_t and tail_norm > 1) else 0)
                    if np_pieces:
                        pw = m // np_pieces
                        for p in range(np_pieces):
                            nc.vector.tensor_scalar_mul(
                                o_t[:, ts(p, pw)], s_tiles[t][:, ts(p, pw)],
                                recs[:, t : t + 1],
                            )
                            if not no_store:
                                (sq if p % 2 == 0 else
                                 (nc.gpsimd if store_q == "sync" else nc.sync)
                                 ).dma_start(out=out[ts(t, 128), ts(p, pw)],
                                             in_=o_t[:, ts(p, pw)])
                        continue
                    nc.vector.tensor_scalar_mul(
                        o_t[:, :], s_tiles[t][:, :], recs[:, t : t + 1]
                    )
                    if not no_store:
                        # alternate DGE rings so trigger processing overlaps
                        (sq if t % 2 == 0 else
                         (nc.gpsimd if store_q == "sync" else nc.sync)
                         ).dma_start(out=out[ts(t, 128), :], in_=o_t[:, :])
    nc.compile()
    return nc


def make_in_maps(Q, K):
    Q = np.asarray(Q, dtype=np.float32)
    K = np.asarray(K, dtype=np.float32)
    in_maps = []
    for i in range(N_CORES):
        b, h = divmod(i, N_CORES // B)
        qs = round_f32r(Q[b, h * ROWS : (h + 1) * ROWS])  # [2048, 64]
        ks = round_f32r(K[b])                             # [4096, 64]
        nrows = qs.shape[0]
        ones = np.ones((1, nrows), np.float32)
        k2 = (ks.astype(np.float64) ** 2).sum(1)
        qt_ext = np.concatenate([qs.T, ones], axis=0)
        kt_ext = np.concatenate(
            [ks.T, (-0.5 * k2)[None, :].astype(np.float32)], axis=0)
        q2 = (qs.astype(np.float64) ** 2).sum(1)  # [2048]
        q2c = (C_SCALE * C_SCALE * q2).astype(np.float32).reshape(-1, 128).T
        in_maps.append({
            "qt": np.ascontiguousarray(qt_ext),
            "kt": np.ascontiguousarray(kt_ext),
            "q2c": np.ascontiguousarray(q2c),
        })
    return in_maps


_NC_CACHE = {}


def get_nc(**kw):
    key = tuple(sorted((k, tuple(v) if isinstance(v, (list, tuple)) else v)
                       for k, v in kw.items()))
    if key not in _NC_CACHE:
        _NC_CACHE[key] = build_kernel(**kw)
    return _NC_CACHE[key]


def kernel(Q, K):
    nc = get_nc(**DEFAULT_KW)
    in_maps = make_in_maps(Q, K)
    res = run_bass_kernel_spmd(nc, in_maps, core_ids=list(range(N_CORES)))
    out = np.empty((B, N, M), dtype=np.float32)
    for i in range(N_CORES):
        b, h = divmod(i, N_CORES // B)
        out[b, h * ROWS : (h + 1) * ROWS] = res.results[i]["out"].astype(np.float32)
    return out
